# revision 21
# baseline (speedup 1.0000x reference)
"""Trainium2 Bass kernel for a dense transformer block (B=4, N=2048, C=768, H=12).

Sharding: 8 cores = 4 batches x 2 sequence halves; each core's batch rows are
rolled so its own 1024 query rows are rows 0:1023 (softmax is permutation-
invariant over keys). Each core computes LN1 over all 2048 rows, full K/V,
attention + MLP for its own 1024 rows. No collectives.

All GEMMs run in fp8 (e4m3) DoubleRow perf mode: operands are [128, 2, *]
access patterns holding two stacked 128-deep contraction tiles (contraction
index c = p + 128*j + 256*instr), which streams 2 output columns per cycle
(4x the f32r rate). Weights are quantized+packed host-side with power-of-two
scales centering fp8's dynamic range; descales fold into activation scale
arguments and output tensor_scalar copies. Softmax: exp on ACT writes fp8
probability tiles that feed the attn@V DoubleRow matmul directly; the
denominator rides a 1-column DoubleRow matmul against a constant tile so each
head's accumulators fit single PSUM banks. Transposes run in bf16 on the PE.

Scheduling: engines execute in-order, so emission order is tuned to start the
softmax exp stream (the global bottleneck: ~25M exps/core on ACT) as early as
possible and keep it dense: LN1 uses a resident bf16 copy of x (no
slot-reuse stalls), attention runs in head-pair batches with the key loop
outermost, V projections and the second-half K matmuls are emitted inside
batch 0's key loop, and the handful of copies the first batch depends on run
on ACT (idle until exp starts) while the rest go to DVE/GPSIMD.
"""

import numpy as np
import ml_dtypes

B, N, C = 4, 2048, 768
H, DH = 12, 64
HID = 4 * C
SCALE = DH ** -0.5
EPS = 1e-5

P = 128
NO = 1024           # own query rows per core
NT = N // P         # 16 token tiles
NOT_ = NO // P      # 8 own token tiles

FP8 = ml_dtypes.float8_e4m3
BF16NP = ml_dtypes.bfloat16

WQ_S = 16.0 * SCALE ** 0.5   # on w_q and w_k (scores psum = 256*SCALE*qk)
WV_S = 16.0                  # on w_v; denominator ones are 16 so it cancels
WP_S = 16.0
Y_S = 32.0                   # on normalized attention output
W1_S = 16.0
W2_S = 32.0


def _build_bass(ln1_affine, ln2_affine):
    import concourse.bass as bass
    import concourse.tile as tile
    from concourse import bacc, mybir
    from concourse.masks import make_identity
    from concourse.alu_op_type import AluOpType as A

    F32 = mybir.dt.float32
    F8 = mybir.dt.float8e4
    B16 = mybir.dt.bfloat16
    AF = mybir.ActivationFunctionType
    PM = mybir.MatmulPerfMode
    DR = PM.DoubleRow

    nc = bacc.Bacc("TRN2", target_bir_lowering=False, num_swdge_queues=4)

    xb = nc.dram_tensor("xb", [N, C], F32, kind="ExternalInput")
    xb16 = nc.dram_tensor("xb16", [N, C], B16, kind="ExternalInput")
    wq_p = nc.dram_tensor("wq_p", [P, 3, 2, C], F8, kind="ExternalInput")
    wk_p = nc.dram_tensor("wk_p", [P, 3, 2, C], F8, kind="ExternalInput")
    wv_p = nc.dram_tensor("wv_p", [P, 3, 2, C], F8, kind="ExternalInput")
    wp_p = nc.dram_tensor("wp_p", [P, 3, 2, C], F8, kind="ExternalInput")
    w1_p = nc.dram_tensor("w1_p", [P, 3, 2, HID], F8, kind="ExternalInput")
    w2_p = nc.dram_tensor("w2_p", [P, 12, 2, C], F8, kind="ExternalInput")
    bp_s = nc.dram_tensor("bp_s", [C], F32, kind="ExternalInput")    # 512*b_proj
    bf1_d = nc.dram_tensor("bf1_d", [HID], F32, kind="ExternalInput")
    bf2_s = nc.dram_tensor("bf2_s", [C], F32, kind="ExternalInput")  # 32*b_fc2
    if ln1_affine:
        ln1_g = nc.dram_tensor("ln1_g", [C], F32, kind="ExternalInput")
        ln1_b = nc.dram_tensor("ln1_b", [C], F32, kind="ExternalInput")
    if ln2_affine:
        ln2_g = nc.dram_tensor("ln2_g", [C], F32, kind="ExternalInput")
        ln2_b = nc.dram_tensor("ln2_b", [C], F32, kind="ExternalInput")
    out_d = nc.dram_tensor("out", [NO, C], F32, kind="ExternalOutput")

    dma = nc.sync.dma_start    # SP-engine HWDGE: keeps Pool free for compute

    with tile.TileContext(nc) as tc:
        consts = tc.alloc_tile_pool(name="consts", bufs=1)
        pers = tc.alloc_tile_pool(name="pers", bufs=1)
        work = tc.alloc_tile_pool(name="work", bufs=2)
        attn = tc.alloc_tile_pool(name="attn", bufs=1)

        identf = consts.tile([P, P], F32)
        make_identity(nc, identf)
        identb = consts.tile([P, P], B16)
        nc.vector.tensor_copy(identb, identf)
        eps_t = consts.tile([P, 1], F32)
        nc.vector.memset(eps_t, EPS)
        ones8 = consts.tile([P, 2, 1], F8)
        nc.vector.memset(ones8, 16.0)
        bpT = consts.tile([P, 6], F32)
        bf1T = consts.tile([P, 24], F32)
        bf2T = consts.tile([P, 6], F32)

        # ---- persistent tiles
        x2 = pers.tile([P, NOT_, C], F32)
        h2T = pers.tile([P, 6, NO], F8)
        yT = pers.tile([P, 3, 2, NO], F8)
        mvs = pers.tile([P, NT, 2], F32)
        rsig = pers.tile([P, NT], F32)
        mvs2 = pers.tile([P, NOT_, 2], F32)
        rsig2 = pers.tile([P, NOT_], F32)
        w1_t = pers.tile([P, 3, 2, HID], F8)
        w2_t = pers.tile([P, 12, 2, C], F8)

        # ---- attention-phase tiles
        xbt = attn.tile([P, NT, C], B16)     # resident bf16 x for LN1
        hT = attn.tile([P, 6, N], F8, tag="hT")
        QT = [attn.tile([96, 2, NO], F8, name="QT%d" % g) for g in range(4)]
        KT = [attn.tile([96, 2, N], F8, name="KT%d" % g) for g in range(4)]
        Vp = attn.tile([P, NT // 2, 2, H, 64], F8)
        y_sb = [attn.tile([P, NOT_, DH], B16, name="ysb%d" % h) for h in range(H)]
        wq_t = attn.tile([P, 3, 2, C], F8, tag="wq")
        wk_t = attn.tile([P, 3, 2, C], F8)
        wv_t = attn.tile([P, 3, 2, C], F8)

        if ln1_affine:
            g1_bc = attn.tile([P, C], F32)
            dma(out=g1_bc, in_=ln1_g[:].partition_broadcast(P))
            b1_bc = attn.tile([P, C], F32)
            dma(out=b1_bc, in_=ln1_b[:].partition_broadcast(P))
        if ln2_affine:
            g2_bc = pers.tile([P, C], F32)
            dma(out=g2_bc, in_=ln2_g[:].partition_broadcast(P))
            b2_bc = pers.tile([P, C], F32)
            dma(out=b2_bc, in_=ln2_b[:].partition_broadcast(P))

        def ln_stats(xt, mvt, i):
            st = work.tile([P, 3, 6], F32, tag="st")
            for s in range(3):
                nc.vector.bn_stats(out=st[:, s, :], in_=xt[:, s * 256:(s + 1) * 256])
            nc.vector.bn_aggr(out=mvt[:, i, :], in_=st)

        def ln_rsqrt(mvt, rst, sl):
            lnv = work.tile([P, 8], F32, tag="lnv")
            n = sl.stop - sl.start
            nc.scalar.activation(out=lnv[:, 0:n], in_=mvt[:, sl, 1], func=AF.Ln,
                                 bias=eps_t)
            nc.scalar.activation(out=rst[:, sl], in_=lnv[:, 0:n], func=AF.Exp,
                                 scale=-0.5)

        def ln_normalize(xt, mvt, rst, i, g_bc, b_bc):
            # on GPSIMD: frees DVE, all-SBUF operands
            hb = work.tile([P, C], B16, tag="hb", bufs=2)
            with nc.allow_low_precision(reason="bf16 ln out"):
                nc.gpsimd.tensor_scalar(out=hb, in0=xt,
                                        scalar1=mvt[:, i, 0:1],
                                        scalar2=rst[:, i:i + 1],
                                        op0=A.subtract, op1=A.mult)
                if g_bc is not None:
                    nc.gpsimd.tensor_tensor(out=hb, in0=hb, in1=g_bc, op=A.mult)
                if b_bc is not None:
                    nc.gpsimd.tensor_tensor(out=hb, in0=hb, in1=b_bc, op=A.add)
            return hb

        # =========== Phase A + B: LN1 -> fp8 hT; QKV; attention
        with tc.tile_pool(name="ps_a", bufs=2, space="PSUM") as ps_a, \
             tc.tile_pool(name="ps_b", bufs=1, space="PSUM") as ps_b:

            for q4 in range(4):
                dma(out=xbt[:, 4 * q4:4 * q4 + 4, :],
                    in_=xb16[q4 * 512:(q4 + 1) * 512, :]
                    .rearrange("(i p) c -> p i c", p=P))

            def phase_a_half(half):
                for k in range(NOT_):
                    ln_stats(xbt[:, 8 * half + k, :], mvs, 8 * half + k)
                ln_rsqrt(mvs, rsig, slice(8 * half, 8 * half + 8))
                for k in range(NOT_):
                    i = 8 * half + k
                    hb = ln_normalize(xbt[:, i, :], mvs, rsig, i,
                                      g1_bc if ln1_affine else None,
                                      b1_bc if ln1_affine else None)
                    tp = ps_a.tile([P, C], B16, tag="tr")
                    for t in range(6):
                        nc.tensor.transpose(tp[:, t * P:(t + 1) * P],
                                            hb[:, t * P:(t + 1) * P], identb)
                    with nc.allow_low_precision(reason="fp8 hT"):
                        nc.vector.tensor_copy(
                            out=hT[:, :, i * P:(i + 1) * P],
                            in_=tp[:].rearrange("p (t n) -> p t n", t=6))

            def copy_qk(eng, dst, src):
                with nc.allow_low_precision(reason="fp8 qk pack"):
                    if eng == "act":
                        nc.scalar.copy(out=dst, in_=src)
                    else:
                        nc.vector.tensor_copy(out=dst, in_=src)

            def emit_q(g):
                # g0's copies on ACT (idle pre-exp); others on DVE
                eng = "act" if g == 0 else "dve"
                for dj in range(2):
                    cw = 96 * (2 * g + dj)
                    for ch2 in range(2):
                        qp = ps_b.tile([P, 512], F32, tag="qk", bufs=2)
                        for ci in range(3):
                            nc.tensor.matmul(
                                qp[0:96, :], wq_t[:, ci, :, cw:cw + 96],
                                hT[:, 2 * ci:2 * ci + 2, ch2 * 512:(ch2 + 1) * 512],
                                start=(ci == 0), stop=(ci == 2), perf_mode=DR)
                        copy_qk(eng, QT[g][:, dj, ch2 * 512:(ch2 + 1) * 512],
                                qp[0:96, :])

            def emit_k(g, ch4):
                eng = "act" if g == 0 else "dve"
                for dj in range(2):
                    cw = 96 * (2 * g + dj)
                    kp = ps_b.tile([P, 512], F32, tag="qk", bufs=2)
                    for ci in range(3):
                        nc.tensor.matmul(
                            kp[0:96, :], wk_t[:, ci, :, cw:cw + 96],
                            hT[:, 2 * ci:2 * ci + 2, ch4 * 512:(ch4 + 1) * 512],
                            start=(ci == 0), stop=(ci == 2), perf_mode=DR)
                    copy_qk(eng, KT[g][:, dj, ch4 * 512:(ch4 + 1) * 512],
                            kp[0:96, :])

            def emit_v(i0, i1):
                # V [token, vdim] tiles -> paired Vp layout
                for i in range(i0, i1):
                    for pg in range(3):
                        vp = ps_b.tile([P, 256], F32, tag="v", bufs=2)
                        for ci in range(3):
                            nc.tensor.matmul(
                                vp, hT[:, 2 * ci:2 * ci + 2, i * P:(i + 1) * P],
                                wv_t[:, ci, :, 256 * pg:256 * (pg + 1)],
                                start=(ci == 0), stop=(ci == 2), perf_mode=DR)
                        with nc.allow_low_precision(reason="fp8 v"):
                            nc.vector.tensor_copy(
                                out=Vp[:, i // 2, i % 2, 4 * pg:4 * pg + 4, :],
                                in_=vp[:].rearrange("p (h d) -> p h d", h=4))

            dma(out=wq_t, in_=wq_p[:, :, :, :])
            dma(out=wk_t, in_=wk_p[:, :, :, :])
            phase_a_half(0)
            dma(out=wv_t, in_=wv_p[:, :, :, :])
            emit_q(0)
            emit_k(0, 0)
            emit_k(0, 1)
            phase_a_half(1)
            dma(out=bpT, in_=bp_s[:].rearrange("(t p) -> p t", p=P))
            dma(out=bf1T, in_=bf1_d[:].rearrange("(t p) -> p t", p=P))
            dma(out=bf2T, in_=bf2_s[:].rearrange("(t p) -> p t", p=P))
            emit_k(0, 2)
            emit_k(0, 3)
            emit_q(1)
            for c4 in range(4):
                emit_k(1, c4)
            emit_v(0, NT)
            for g2 in (2, 3):
                emit_q(g2)
                for c4 in range(4):
                    emit_k(g2, c4)
            # wp reuses wq's slot (wq last read by the Q matmuls above)
            wp_t = attn.tile([P, 3, 2, C], F8, tag="wq")
            dma(out=wp_t, in_=wp_p[:, :, :, :])

        # ---- attention: head pairs, key-pair loop outermost; the exp stream
        # on ACT is the pacer, everything else has slack
        with tc.tile_pool(name="ps_s", bufs=1, space="PSUM") as ps_s:
                for hb2 in range(6):
                    h0, h1 = 2 * hb2, 2 * hb2 + 1
                    ya = [ps_s.tile([P, NOT_, DH], F32, tag="ya%d" % hh,
                                    name="ya_%d" % (2 * hb2 + hh))
                          for hh in range(2)]
                    dn = ps_s.tile([P, 2, NOT_], F32, tag="dn",
                                   name="dn_%d" % hb2)
                    for mp in range(NT // 2):
                        for hh, hx in enumerate((h0, h1)):
                            gx, pox = hx // 3, 32 * (hx % 3)
                            eA = work.tile([P, 2, NO], F8, tag="eA", bufs=3)
                            for sub in range(2):
                                m = 2 * mp + sub
                                sp = ps_s.tile([P, NO], F32, tag="s", bufs=2)
                                for ch2 in range(2):
                                    nc.tensor.matmul(
                                        sp[:, ch2 * 512:(ch2 + 1) * 512],
                                        KT[gx][pox:pox + 32, :, m * P:(m + 1) * P],
                                        QT[gx][pox:pox + 32, :,
                                               ch2 * 512:(ch2 + 1) * 512],
                                        start=True, stop=True, perf_mode=DR)
                                with nc.allow_low_precision(reason="fp8 probs"):
                                    nc.scalar.activation(
                                        out=eA[:, sub, :], in_=sp,
                                        func=AF.Exp, scale=1.0 / 256.0)
                            st_f = (mp == 0)
                            sp_f = (mp == NT // 2 - 1)
                            for qt in range(NOT_):
                                nc.tensor.matmul(
                                    ya[hh][:, qt, :],
                                    eA[:, :, qt * P:(qt + 1) * P],
                                    Vp[:, mp, :, hx, :],
                                    start=st_f, stop=sp_f, perf_mode=DR)
                                nc.tensor.matmul(
                                    dn[:, hh, qt:qt + 1],
                                    eA[:, :, qt * P:(qt + 1) * P], ones8,
                                    start=st_f, stop=sp_f, perf_mode=DR)
                    rin = work.tile([P, 2, NOT_], F32, tag="rin")
                    with nc.allow_low_precision(reason="softmax denom recip"):
                        nc.vector.reciprocal(rin, dn)
                    for hh in range(2):
                        for qt in range(NOT_):
                            with nc.allow_low_precision(reason="bf16 y"):
                                nc.vector.tensor_scalar(
                                    out=y_sb[2 * hb2 + hh][:, qt, :],
                                    in0=ya[hh][:, qt, :],
                                    scalar1=rin[:, hh, qt:qt + 1], scalar2=Y_S,
                                    op0=A.mult, op1=A.mult)

        # MLP weights arrive during attention
        dma(out=w1_t, in_=w1_p[:, :, :, :])
        dma(out=w2_t, in_=w2_p[:, :, :, :])

        # =========== Phase D: y transposes + output projection
        # attnT reuses hT's 12KB slot (hT last read by the V matmuls)
        attnT = attn.tile([P, 6, NO], B16, tag="hT")
        with tc.tile_pool(name="ps_p", bufs=2, space="PSUM") as ps_p:
            for i3 in range(3):
                for jj in range(2):
                    ha, hx = 4 * i3 + 2 * jj, 4 * i3 + 2 * jj + 1
                    for qt in range(NOT_):
                        ytp = ps_p.tile([P, P], B16, tag="yt")
                        nc.tensor.transpose(ytp[0:64, :], y_sb[ha][:, qt, :],
                                            identb)
                        nc.tensor.transpose(ytp[64:128, :], y_sb[hx][:, qt, :],
                                            identb)
                        with nc.allow_low_precision(reason="fp8 yT"):
                            nc.vector.tensor_copy(
                                out=yT[:, i3, jj, qt * P:(qt + 1) * P], in_=ytp)
            for jt in range(6):
                for ch2 in range(2):
                    pp = ps_p.tile([P, 512], F32, tag="pp")
                    for ci in range(3):
                        nc.tensor.matmul(
                            pp, wp_t[:, ci, :, jt * P:(jt + 1) * P],
                            yT[:, ci, :, ch2 * 512:(ch2 + 1) * 512],
                            start=(ci == 0), stop=(ci == 2), perf_mode=DR)
                    with nc.allow_low_precision(reason="bf16 attnT"):
                        nc.vector.tensor_scalar(
                            out=attnT[:, jt, ch2 * 512:(ch2 + 1) * 512], in0=pp,
                            scalar1=bpT[:, jt:jt + 1], scalar2=1.0 / 512.0,
                            op0=A.add, op1=A.mult)

        # =========== Phase E: residual + LN2 -> h2T
        def load_x_chunk(c0, ntile, tag="xf"):
            xt = work.tile([P, ntile, C], F32, tag=tag, bufs=2)
            dma(out=xt, in_=xb[c0 * P:(c0 + ntile) * P, :]
                .rearrange("(i p) c -> p i c", p=P))
            return xt

        with tc.tile_pool(name="ps_e", bufs=3, space="PSUM") as ps_e:
            for i in range(NOT_):
                if i % 2 == 0:
                    xo = load_x_chunk(i, 2, tag="xf")
                tr = ps_e.tile([P, C], B16, tag="tr")
                for t in range(6):
                    nc.tensor.transpose(tr[:, t * P:(t + 1) * P],
                                        attnT[:, t, i * P:(i + 1) * P], identb)
                nc.vector.tensor_tensor(out=x2[:, i, :], in0=tr,
                                        in1=xo[:, i % 2, :], op=A.add)
                ln_stats(x2[:, i, :], mvs2, i)
            ln_rsqrt(mvs2, rsig2, slice(0, NOT_))
            for i in range(NOT_):
                hb2 = ln_normalize(x2[:, i, :], mvs2, rsig2, i,
                                   g2_bc if ln2_affine else None,
                                   b2_bc if ln2_affine else None)
                tp2 = ps_e.tile([P, C], B16, tag="tr")
                for t in range(6):
                    nc.tensor.transpose(tp2[:, t * P:(t + 1) * P],
                                        hb2[:, t * P:(t + 1) * P], identb)
                with nc.allow_low_precision(reason="fp8 h2T"):
                    nc.vector.tensor_copy(
                        out=h2T[:, :, i * P:(i + 1) * P],
                        in_=tp2[:].rearrange("p (t n) -> p t n", t=6))

        attn.release()

        # =========== Phase F: MLP per 512-token half + output
        for nh in range(2):
            sl = slice(nh * 512, (nh + 1) * 512)
            with tc.tile_pool(name="ps_m%d" % nh, bufs=1, space="PSUM") as ps_m:
                f2s = [ps_m.tile([P, 512], F32, tag="f2c%d" % jt,
                                 name="f2acc%d_%d" % (nh, jt))
                       for jt in range(6)]

                def emit_fc1(t2):
                    gh = work.tile([P, 2, 512], F8, tag="ghat", bufs=2)
                    for j2 in range(2):
                        fps = ps_m.tile([P, 512], F32, tag="f1", bufs=2)
                        hw = P * (2 * t2 + j2)
                        for ci in range(3):
                            nc.tensor.matmul(
                                fps, w1_t[:, ci, :, hw:hw + P],
                                h2T[:, 2 * ci:2 * ci + 2, sl],
                                start=(ci == 0), stop=(ci == 2), perf_mode=DR)
                        with nc.allow_low_precision(reason="fp8 gelu"):
                            nc.scalar.activation(
                                out=gh[:, j2, :], in_=fps, func=AF.Gelu,
                                scale=1.0 / W1_S,
                                bias=bf1T[:, 2 * t2 + j2:2 * t2 + j2 + 1])
                    return gh

                # software pipeline: fc1(t2+1) emitted before fc2(t2) so the
                # PE never stalls waiting on gelu
                gh_cur = emit_fc1(0)
                for t2 in range(12):
                    gh_next = emit_fc1(t2 + 1) if t2 < 11 else None
                    for jt in range(6):
                        nc.tensor.matmul(
                            f2s[jt], w2_t[:, t2, :, jt * P:(jt + 1) * P],
                            gh_cur,
                            start=(t2 == 0), stop=(t2 == 11), perf_mode=DR)
                    gh_cur = gh_next
                mlpT = work.tile([P, 6, 512], B16, tag="mlpT", bufs=1)
                for jt in range(6):
                    with nc.allow_low_precision(reason="bf16 mlpT"):
                        nc.vector.tensor_scalar(
                            out=mlpT[:, jt, :], in0=f2s[jt],
                            scalar1=bf2T[:, jt:jt + 1], scalar2=1.0 / W2_S,
                            op0=A.add, op1=A.mult)
            with tc.tile_pool(name="ps_o%d" % nh, bufs=2, space="PSUM") as ps_o:
                for it in range(4):
                    i = 4 * nh + it
                    tro = ps_o.tile([P, C], B16, tag="tro")
                    for t in range(6):
                        nc.tensor.transpose(tro[:, t * P:(t + 1) * P],
                                            mlpT[:, t, it * P:(it + 1) * P],
                                            identb)
                    o_sb = work.tile([P, C], F32, tag="osb", bufs=2)
                    nc.vector.tensor_tensor(out=o_sb, in0=tro, in1=x2[:, i, :],
                                            op=A.add)
                    dma(out=out_d[i * P:(i + 1) * P, :], in_=o_sb)

        work.release()
        pers.release()
        consts.release()

    nc.compile()
    return nc


def _pack_dr(w):
    """[C_contract, cols] -> [128, C/256, 2, cols] with c = p + 128*j + 256*i."""
    ctr, cols = w.shape
    return np.ascontiguousarray(
        w.reshape(ctr // 256, 2, 128, cols).transpose(2, 0, 1, 3))


def _reorder_qk(w):
    """Reorder head-dim cols: (tg, h3, dj, p32) -> (tg, dj, h3, p32)."""
    return np.ascontiguousarray(
        w.reshape(C, 4, 3, 2, 32).transpose(0, 1, 3, 2, 4).reshape(C, C))


_NC_CACHE = None
_NC_KEY = None


def kernel(x, ln1_g, ln1_b, w_qkv, w_proj, b_proj, ln2_g, ln2_b,
           w_fc1, b_fc1, w_fc2, b_fc2):
    global _NC_CACHE, _NC_KEY
    from concourse.bass_utils import run_bass_kernel_spmd

    x = np.asarray(x, np.float32)
    w_qkv = np.asarray(w_qkv, np.float32)
    ln1_g = np.asarray(ln1_g, np.float32)
    ln1_b = np.asarray(ln1_b, np.float32)
    ln2_g = np.asarray(ln2_g, np.float32)
    ln2_b = np.asarray(ln2_b, np.float32)

    ln1_affine = not (np.all(ln1_g == 1.0) and np.all(ln1_b == 0.0))
    ln2_affine = not (np.all(ln2_g == 1.0) and np.all(ln2_b == 0.0))

    wq = _reorder_qk(w_qkv[:, 0:C]) * WQ_S
    wk = _reorder_qk(w_qkv[:, C:2 * C]) * WQ_S
    wv = w_qkv[:, 2 * C:3 * C] * WV_S

    shared = {
        "wq_p": _pack_dr(wq).astype(FP8),
        "wk_p": _pack_dr(wk).astype(FP8),
        "wv_p": _pack_dr(wv).astype(FP8),
        "wp_p": _pack_dr(np.asarray(w_proj, np.float32) * WP_S).astype(FP8),
        "w1_p": _pack_dr(np.asarray(w_fc1, np.float32) * W1_S).astype(FP8),
        "w2_p": _pack_dr(np.asarray(w_fc2, np.float32) * W2_S).astype(FP8),
        "bp_s": np.asarray(b_proj, np.float32) * 512.0,
        "bf1_d": np.asarray(b_fc1, np.float32),
        "bf2_s": np.asarray(b_fc2, np.float32) * W2_S,
    }
    if ln1_affine:
        shared["ln1_g"] = ln1_g
        shared["ln1_b"] = ln1_b
    if ln2_affine:
        shared["ln2_g"] = ln2_g
        shared["ln2_b"] = ln2_b

    key = (ln1_affine, ln2_affine)
    if _NC_CACHE is None or _NC_KEY != key:
        _NC_CACHE = _build_bass(ln1_affine, ln2_affine)
        _NC_KEY = key

    in_maps = []
    for c in range(8):
        b, hh = c // 2, c % 2
        xbv = np.ascontiguousarray(np.roll(x[b], -hh * NO, axis=0))
        in_maps.append({"xb": xbv, "xb16": xbv.astype(BF16NP), **shared})

    res = run_bass_kernel_spmd(_NC_CACHE, in_maps, core_ids=list(range(8)))

    outp = np.empty((B, N, C), np.float32)
    for c in range(8):
        b, hh = c // 2, c % 2
        outp[b, hh * NO:(hh + 1) * NO, :] = res.results[c]["out"]
    return outp
